# revision 1
# baseline (speedup 1.0000x reference)
"""Dense GAT (2-layer, 8+1 heads) on 8 Trainium2 NeuronCores — V2.

Row-parallel over destination rows. Per core:
  - adjT built once on PE from an fp16 adj row-slice, reused by both layers.
  - h1|a_src1 from one matmul chain against host-folded [w1 | w1@blockdiag(att_src1)].
  - E^T[j,i] tiles produced by one of two chains, split to balance engines:
      F1 (ACT): t = Prelu(ad_b + a_src[j]); e = exp(t) * adjT     [2 ACT + 1 DVE]
      F3 (DVE): e = adjT * max(exp(as[j])exp(ad[i]), exp(.2as[j])exp(.2ad[i]))
                 via tensor_scalar + scalar_tensor_tensor          [3 DVE]
  - softmax denominators ride as a ones column in the aggregation lhsT.
  - one AllGather of [512, 132] (h2 | a2 | exp(a_src2) | exp(.2 a_src2)).
E-path tensors are fp16 (values O(1..20)); accumulation stays fp32 in PSUM.
"""
import numpy as np

N = 4096
F_IN = 256
HID = 64
H1 = 8
F1 = H1 * HID
OUT = 128
N_CORES = 8
R = N // N_CORES
JT = N // 128
IT = R // 128
NEG_ATT = 0.2
NEG_OUT = 0.01

# chain assignment tuning: F3 if ((h * JT + jt) * 3 + 1) % 16 < F3_16THS
F3_16THS = 7
L2_F3_16THS = 8

_CACHE = {}


def _is_f3(h, jt):
    return ((h * JT + jt) * 3 + 1) % 16 < F3_16THS


def _is_f3_l2(jt):
    return ((jt) * 5 + 3) % 16 < L2_F3_16THS


def _build():
    import concourse.bass as bass
    from concourse import bacc
    import concourse.mybir as mybir
    import concourse.tile as tile
    from concourse.masks import make_identity

    f32 = mybir.dt.float32
    f16 = mybir.dt.float16
    A = mybir.ActivationFunctionType
    Al = mybir.AluOpType

    nc = bacc.Bacc("TRN2", target_bir_lowering=False, debug=False,
                   num_devices=N_CORES)
    d_xT = nc.dram_tensor("xT", [F_IN, N], f32, kind="ExternalInput")
    d_xmT = nc.dram_tensor("xmT", [F_IN, R], f32, kind="ExternalInput")
    d_adjr = nc.dram_tensor("adjr", [R, N], f16, kind="ExternalInput")
    d_rhs1 = nc.dram_tensor("rhs1", [F_IN, F1 + H1], f32, kind="ExternalInput")
    d_vdst1 = nc.dram_tensor("vdst1", [F_IN, H1], f32, kind="ExternalInput")
    d_rhs2 = nc.dram_tensor("rhs2", [F1, OUT + 2], f32, kind="ExternalInput")
    d_b1c = nc.dram_tensor("b1c", [HID, H1], f32, kind="ExternalInput")
    d_b2c = nc.dram_tensor("b2c", [OUT, 1], f32, kind="ExternalInput")
    d_outT = nc.dram_tensor("outT", [OUT, R], f32, kind="ExternalOutput")

    with tile.TileContext(nc) as tc:
        with tc.tile_pool(name="const", bufs=1) as const, \
             tc.tile_pool(name="big", bufs=1) as big, \
             tc.tile_pool(name="work", bufs=3) as work, \
             tc.tile_pool(name="dram", bufs=1, space="DRAM") as dram, \
             tc.tile_pool(name="ps_tr", bufs=2, space="PSUM") as ps_tr, \
             tc.tile_pool(name="ps_mm", bufs=2, space="PSUM") as ps_mm, \
             tc.tile_pool(name="ps_sm", bufs=2, space="PSUM") as ps_sm, \
             tc.tile_pool(name="ps_ag", bufs=2, space="PSUM") as ps_ag:
            ident = const.tile([128, 128], f32)
            make_identity(nc, ident)
            ident16 = const.tile([128, 128], f16)
            nc.vector.tensor_copy(ident16, ident)
            ones_row = const.tile([1, 128], f32)
            nc.vector.memset(ones_row, 1.0)
            ones_row16 = const.tile([1, 128], f16)
            nc.vector.memset(ones_row16, 1.0)
            ones_col16 = const.tile([128, 1], f16)
            nc.vector.memset(ones_col16, 1.0)
            rhs1_sb = const.tile([128, 2, F1 + H1], f32)
            nc.sync.dma_start(out=rhs1_sb[:, 0, :], in_=d_rhs1[0:128, :])
            nc.sync.dma_start(out=rhs1_sb[:, 1, :], in_=d_rhs1[128:256, :])
            w1_16 = const.tile([128, 2, F1], f16)
            nc.vector.tensor_copy(w1_16[:, 0, :], rhs1_sb[:, 0, 0:F1])
            nc.vector.tensor_copy(w1_16[:, 1, :], rhs1_sb[:, 1, 0:F1])
            vdst1_sb = const.tile([128, 2, H1], f32)
            nc.sync.dma_start(out=vdst1_sb[:, 0, :], in_=d_vdst1[0:128, :])
            nc.sync.dma_start(out=vdst1_sb[:, 1, :], in_=d_vdst1[128:256, :])
            rhs2_sb = const.tile([128, 4, OUT + 2], f32)
            for kt in range(4):
                nc.sync.dma_start(out=rhs2_sb[:, kt, :],
                                  in_=d_rhs2[kt * 128:(kt + 1) * 128, :])
            b1_sb = const.tile([HID, H1], f32)
            nc.sync.dma_start(out=b1_sb, in_=d_b1c[:, :])
            b2_sb = const.tile([OUT, 1], f32)
            nc.sync.dma_start(out=b2_sb, in_=d_b2c[:, :])

            adjT_all = big.tile([128, JT, R], f16)        # 32 KB/part
            h1_all = big.tile([128, JT, H1, HID + 1], f16)  # 32.5 KB/part
            asrc_all = big.tile([128, JT, H1], f32)
            easrc_all = big.tile([128, JT, H1], f16)
            e2src_all = big.tile([128, JT, H1], f32)
            x2T_all = big.tile([128, 4, R], f32)
            adstT = big.tile([H1, R], f32)
            adst_rows = big.tile([1, H1, R], f32)
            adst2T = big.tile([1, R], f32)

            nc.vector.memset(h1_all[:, :, :, HID:HID + 1], 1.0)

            # ---- adjT ----
            adjr_sb = big.tile([128, IT, N], f16)        # 32 KB/part
            for it in range(IT):
                nc.sync.dma_start(out=adjr_sb[:, it, :],
                                  in_=d_adjr[it * 128:(it + 1) * 128, :])
            for jb in range(JT):
                for it in range(IT):
                    ps_t = ps_tr.tile([128, 128], f16, tag="tr16")
                    nc.tensor.transpose(
                        ps_t, adjr_sb[:, it, jb * 128:(jb + 1) * 128], ident16)
                    if (jb * IT + it) % 4 == 0:
                        nc.vector.tensor_copy(
                            adjT_all[:, jb, it * 128:(it + 1) * 128], ps_t)
                    else:
                        nc.scalar.copy(adjT_all[:, jb, it * 128:(it + 1) * 128],
                                       ps_t)

            # ---- a_dst (own rows) ----
            xmT_sb = big.tile([128, 2, R], f32)
            nc.sync.dma_start(out=xmT_sb[:, 0, :], in_=d_xmT[0:128, :])
            nc.sync.dma_start(out=xmT_sb[:, 1, :], in_=d_xmT[128:256, :])
            for it in range(IT):
                ps_ad = ps_sm.tile([128, H1], f32, tag="sm")
                for kb in range(2):
                    nc.tensor.matmul(ps_ad,
                                     xmT_sb[:, kb, it * 128:(it + 1) * 128],
                                     vdst1_sb[:, kb, :],
                                     start=(kb == 0), stop=(kb == 1))
                adm = work.tile([128, H1], f32, tag="adm", bufs=2)
                nc.vector.tensor_copy(adm, ps_ad)
                ps_adT = ps_sm.tile([H1, 128], f32, tag="sm")
                nc.tensor.transpose(ps_adT, adm, ident)
                nc.vector.tensor_copy(adstT[:, it * 128:(it + 1) * 128], ps_adT)
            for h in range(H1):
                nc.sync.dma_start(out=adst_rows[:, h, :], in_=adstT[h:h + 1, :])

            # ---- h1 | a_src ----
            for jt in range(JT):
                xt_t = work.tile([128, 2, 128], f32, tag="xt", bufs=3)
                cols = slice(jt * 128, (jt + 1) * 128)
                nc.sync.dma_start(out=xt_t[:, 0, :], in_=d_xT[0:128, cols])
                nc.sync.dma_start(out=xt_t[:, 1, :], in_=d_xT[128:256, cols])
                xt16 = work.tile([128, 2, 128], f16, tag="xt16", bufs=3)
                if jt % 2 == 0:
                    nc.vector.tensor_copy(xt16[:, 0, :], xt_t[:, 0, :])
                    nc.scalar.copy(xt16[:, 1, :], xt_t[:, 1, :])
                else:
                    nc.scalar.copy(xt16[:, 0, :], xt_t[:, 0, :])
                    nc.vector.tensor_copy(xt16[:, 1, :], xt_t[:, 1, :])
                ps_h = ps_mm.tile([128, F1], f32, tag="h")
                ps_a = ps_sm.tile([128, H1], f32, tag="sm")
                for kb in range(2):
                    nc.tensor.matmul(ps_h, xt16[:, kb, :],
                                     w1_16[:, kb, :],
                                     start=(kb == 0), stop=(kb == 1))
                    nc.tensor.matmul(ps_a, xt_t[:, kb, :],
                                     rhs1_sb[:, kb, F1:F1 + H1],
                                     start=(kb == 0), stop=(kb == 1))
                nc.vector.tensor_copy(
                    h1_all[:, jt, :, 0:HID],
                    ps_h.rearrange("p (h c) -> p h c", c=HID))
                nc.vector.tensor_copy(asrc_all[:, jt, :], ps_a)
                if jt % 8 == 7:
                    g = slice(jt - 7, jt + 1)
                    nc.scalar.activation(easrc_all[:, g, :],
                                         asrc_all[:, g, :], A.Exp)
                    nc.scalar.activation(e2src_all[:, g, :],
                                         asrc_all[:, g, :], A.Exp,
                                         scale=NEG_ATT)

            # ---- layer-1 attention (head-pipelined precompute) ----
            def _pre_head(h):
                ps_b = ps_sm.tile([128, R], f32, tag="sm", name=f"ps_b{h}")
                nc.tensor.matmul(ps_b, ones_row, adst_rows[:, h, :],
                                 start=True, stop=True)
                adb_h = work.tile([128, R], f32, tag="adb", bufs=2,
                                  name=f"adb{h}")
                nc.vector.tensor_copy(adb_h, ps_b)
                urow = work.tile([1, R], f16, tag="urow", bufs=2,
                                 name=f"urow{h}")
                nc.scalar.activation(urow, adst_rows[:, h, :], A.Exp)
                prow = work.tile([1, R], f16, tag="prow", bufs=2,
                                 name=f"prow{h}")
                nc.scalar.activation(prow, adst_rows[:, h, :], A.Exp,
                                     scale=NEG_ATT)
                ps_u = ps_sm.tile([128, R], f32, tag="sm", name=f"ps_u{h}")
                nc.tensor.matmul(ps_u, ones_row16, urow, start=True, stop=True)
                ubc_h = work.tile([128, R], f16, tag="ubc", bufs=2,
                                  name=f"ubc{h}")
                nc.vector.tensor_copy(ubc_h, ps_u)
                ps_p = ps_sm.tile([128, R], f32, tag="sm", name=f"ps_p{h}")
                nc.tensor.matmul(ps_p, ones_row16, prow, start=True, stop=True)
                pbc_h = work.tile([128, R], f16, tag="pbc", bufs=2,
                                  name=f"pbc{h}")
                nc.vector.tensor_copy(pbc_h, ps_p)
                return adb_h, ubc_h, pbc_h

            pre = _pre_head(0)
            for h in range(H1):
                adb_h, ubc_h, pbc_h = pre
                if h + 1 < H1:
                    pre = _pre_head(h + 1)
                ps_agg = ps_ag.tile([HID + 1, R], f32, tag="agg")
                for jt in range(JT):
                    e = work.tile([128, R], f16, tag="e", bufs=4)
                    if _is_f3(h, jt):
                        m2 = work.tile([128, R], f16, tag="m2", bufs=3)
                        nc.vector.tensor_scalar_mul(
                            m2, pbc_h, e2src_all[:, jt, h:h + 1])
                        mx = work.tile([128, R], f16, tag="mx", bufs=3)
                        nc.vector.scalar_tensor_tensor(
                            mx, ubc_h, easrc_all[:, jt, h:h + 1], m2,
                            op0=Al.mult, op1=Al.max)
                        nc.vector.tensor_mul(e, mx, adjT_all[:, jt, :])
                    else:
                        t1 = work.tile([128, R], f32, tag="t1", bufs=3)
                        nc.scalar.activation(
                            t1, adb_h, A.Prelu,
                            bias=asrc_all[:, jt, h:h + 1], alpha=NEG_ATT)
                        t2 = work.tile([128, R], f16, tag="t2", bufs=3)
                        nc.scalar.activation(t2, t1, A.Exp)
                        nc.vector.tensor_mul(e, t2, adjT_all[:, jt, :])
                    nc.tensor.matmul(ps_agg, h1_all[:, jt, h, :], e,
                                     start=(jt == 0), stop=(jt == JT - 1))
                rz = work.tile([1, R], f32, tag="rz", bufs=2)
                nc.vector.reciprocal(rz, ps_agg[HID:HID + 1, :])
                ps_rzb = ps_sm.tile([HID, R], f32, tag="sm")
                nc.tensor.matmul(ps_rzb, ones_row[:, 0:HID], rz,
                                 start=True, stop=True)
                rzb = work.tile([HID, R], f32, tag="rzbs", bufs=2)
                nc.vector.tensor_copy(rzb, ps_rzb)
                y_h = work.tile([HID, R], f32, tag="yh", bufs=2)
                nc.vector.tensor_mul(y_h, ps_agg[0:HID, :], rzb)
                po = (h % 2) * HID
                nc.scalar.activation(
                    x2T_all[po:po + HID, h // 2, :], y_h, A.Prelu,
                    bias=b1_sb[:, h:h + 1], alpha=NEG_OUT)

            # ---- layer 2 ----
            G = OUT // 2 + 4  # fp16-packed h2 (64 f32 words) | as2 | ad2 | eas2 | e02as2
            bounce_in = [dram.tile([R // 2, G], f32, name=f"bin{half}")
                         for half in range(2)]
            bounce_out = [dram.tile([N_CORES, R // 2, G], f32,
                                    addr_space="Shared", name=f"bout{half}")
                          for half in range(2)]
            for it in range(IT):
                ps_h2 = ps_mm.tile([128, OUT + 2], f32, tag="h")
                for kt in range(4):
                    nc.tensor.matmul(
                        ps_h2,
                        x2T_all[:, kt, it * 128:(it + 1) * 128],
                        rhs2_sb[:, kt, :],
                        start=(kt == 0), stop=(kt == 3))
                h2m = work.tile([128, G], f32, tag="h2m", bufs=2)
                nc.vector.tensor_copy(h2m[:, 0:OUT // 2].bitcast(f16),
                                      ps_h2[:, 0:OUT])
                nc.vector.tensor_copy(h2m[:, OUT // 2:OUT // 2 + 2],
                                      ps_h2[:, OUT:OUT + 2])
                nc.scalar.activation(h2m[:, OUT // 2 + 2:OUT // 2 + 3],
                                     ps_h2[:, OUT:OUT + 1], A.Exp)
                nc.scalar.activation(h2m[:, OUT // 2 + 3:OUT // 2 + 4],
                                     ps_h2[:, OUT:OUT + 1], A.Exp, scale=NEG_ATT)
                nc.sync.dma_start(
                    out=bounce_in[it // 2][(it % 2) * 128:(it % 2 + 1) * 128, :],
                    in_=h2m)
                ps_adT2 = ps_sm.tile([1, 128], f32, tag="sm")
                nc.tensor.transpose(ps_adT2, h2m[:, OUT // 2 + 1:OUT // 2 + 2],
                                    ident)
                nc.vector.tensor_copy(adst2T[:, it * 128:(it + 1) * 128],
                                      ps_adT2)
            for half in range(2):
                nc.gpsimd.collective_compute(
                    "AllGather",
                    bass.mybir.AluOpType.bypass,
                    replica_groups=[list(range(N_CORES))],
                    ins=[bounce_in[half].opt()],
                    outs=[bounce_out[half].opt()],
                )
            ps_b2 = ps_sm.tile([128, R], f32, tag="sm")
            nc.tensor.matmul(ps_b2, ones_row, adst2T, start=True, stop=True)
            ad2_b = work.tile([128, R], f32, tag="ad2b", bufs=1)
            nc.vector.tensor_copy(ad2_b, ps_b2)
            u2row = work.tile([1, R], f16, tag="urow", bufs=2)
            nc.scalar.activation(u2row, adst2T, A.Exp)
            p2row = work.tile([1, R], f16, tag="prow", bufs=2)
            nc.scalar.activation(p2row, adst2T, A.Exp, scale=NEG_ATT)
            ps_u2 = ps_sm.tile([128, R], f32, tag="sm")
            nc.tensor.matmul(ps_u2, ones_row16, u2row, start=True, stop=True)
            u2bc = work.tile([128, R], f16, tag="u2bc", bufs=1)
            nc.vector.tensor_copy(u2bc, ps_u2)
            ps_p2 = ps_sm.tile([128, R], f32, tag="sm")
            nc.tensor.matmul(ps_p2, ones_row16, p2row, start=True, stop=True)
            p2bc = work.tile([128, R], f16, tag="p2bc", bufs=1)
            nc.vector.tensor_copy(p2bc, ps_p2)

            ps_o2 = ps_ag.tile([128, R], f32, tag="agg")
            ps_z2 = ps_sm.tile([1, R], f32, tag="sm")
            h2g_all = big.tile([128, 2, N_CORES, 2, G], f32)
            for half in range(2):
                for c4 in range(N_CORES):
                    nc.sync.dma_start(
                        out=h2g_all[:, half, c4, :, :],
                        in_=bounce_out[half][c4].rearrange(
                            "(r1 p) g -> p r1 g", p=128))
            jt_order = [c4 * 4 + half * 2 + r2
                        for half in range(2) for c4 in range(N_CORES)
                        for r2 in range(2)]
            for step, jt in enumerate(jt_order):
                half, c4, r2 = (jt % 4) // 2, jt // 4, jt % 2
                h2g = h2g_all[:, half, c4, r2, :]
                h2g16 = h2g[:, 0:OUT // 2].bitcast(f16)
                e2 = work.tile([128, R], f16, tag="e", bufs=4)
                if _is_f3_l2(jt):
                    ea = work.tile([128, 1], f16, tag="ea2", bufs=3)
                    nc.vector.tensor_copy(ea, h2g[:, OUT // 2 + 2:OUT // 2 + 3])
                    m2 = work.tile([128, R], f16, tag="m2", bufs=3)
                    nc.vector.tensor_scalar_mul(m2, p2bc,
                                                h2g[:, OUT // 2 + 3:OUT // 2 + 4])
                    mx = work.tile([128, R], f16, tag="mx", bufs=3)
                    nc.vector.scalar_tensor_tensor(
                        mx, u2bc, ea[:, 0:1], m2, op0=Al.mult, op1=Al.max)
                    nc.vector.tensor_mul(e2, mx, adjT_all[:, jt, :])
                else:
                    t1 = work.tile([128, R], f32, tag="t1", bufs=3)
                    nc.scalar.activation(t1, ad2_b, A.Prelu,
                                         bias=h2g[:, OUT // 2:OUT // 2 + 1],
                                         alpha=NEG_ATT)
                    t2 = work.tile([128, R], f16, tag="t2", bufs=3)
                    nc.scalar.activation(t2, t1, A.Exp)
                    nc.vector.tensor_mul(e2, t2, adjT_all[:, jt, :])
                nc.tensor.matmul(ps_o2, h2g16, e2,
                                 start=(step == 0), stop=(step == JT - 1))
                nc.tensor.matmul(ps_z2, ones_col16, e2,
                                 start=(step == 0), stop=(step == JT - 1))
            rz2 = work.tile([1, R], f32, tag="rz", bufs=2)
            nc.vector.reciprocal(rz2, ps_z2[0:1, :])
            ps_rz2b = ps_sm.tile([128, R], f32, tag="sm")
            nc.tensor.matmul(ps_rz2b, ones_row, rz2, start=True, stop=True)
            rz2b = work.tile([128, R], f32, tag="rz2bs", bufs=1)
            nc.vector.tensor_copy(rz2b, ps_rz2b)
            o2 = work.tile([128, R], f32, tag="o2s", bufs=1)
            nc.vector.tensor_mul(o2, ps_o2, rz2b)
            outT_sb = work.tile([OUT, R], f32, tag="outT", bufs=1)
            nc.scalar.activation(outT_sb, o2, A.Prelu,
                                 bias=b2_sb[:, 0:1], alpha=NEG_OUT)
            nc.sync.dma_start(out=d_outT[:, :], in_=outT_sb)

    nc.finalize()
    return nc


def _prep_host(x, adj, w1, att_src1, att_dst1, b1, w2, att_src2, att_dst2, b2):
    x = np.asarray(x, np.float32).reshape(N, F_IN)
    adj = np.asarray(adj, np.float32).reshape(N, N)
    w1 = np.asarray(w1, np.float32)
    w2 = np.asarray(w2, np.float32)
    att_src1 = np.asarray(att_src1, np.float32)
    att_dst1 = np.asarray(att_dst1, np.float32)
    att_src2 = np.asarray(att_src2, np.float32)
    att_dst2 = np.asarray(att_dst2, np.float32)
    b1 = np.asarray(b1, np.float32)
    b2 = np.asarray(b2, np.float32)

    xT = np.ascontiguousarray(x.T)
    adj16 = adj.astype(np.float16)
    v_src1 = np.empty((F_IN, H1), np.float32)
    v_dst1 = np.empty((F_IN, H1), np.float32)
    for h in range(H1):
        blk = w1[:, h * HID:(h + 1) * HID]
        v_src1[:, h] = blk @ att_src1[h]
        v_dst1[:, h] = blk @ att_dst1[h]
    rhs1 = np.ascontiguousarray(np.concatenate([w1, v_src1], axis=1))
    v_src2 = (w2 @ att_src2[0])[:, None]
    v_dst2 = (w2 @ att_dst2[0])[:, None]
    rhs2 = np.ascontiguousarray(np.concatenate([w2, v_src2, v_dst2], axis=1))
    b1c = np.ascontiguousarray(b1.reshape(H1, HID).T)
    b2c = np.ascontiguousarray(b2.reshape(OUT, 1))

    in_maps = []
    for c in range(N_CORES):
        rows = slice(c * R, (c + 1) * R)
        in_maps.append({
            "xT": xT,
            "xmT": np.ascontiguousarray(xT[:, rows]),
            "adjr": adj16[rows, :],
            "rhs1": rhs1,
            "vdst1": v_dst1,
            "rhs2": rhs2,
            "b1c": b1c,
            "b2c": b2c,
        })
    return in_maps


def kernel(**inputs) -> np.ndarray:
    from concourse.bass_utils import run_bass_kernel_spmd

    if "nc" not in _CACHE:
        _CACHE["nc"] = _build()
    nc = _CACHE["nc"]
    in_maps = _prep_host(**inputs)
    try:
        res = run_bass_kernel_spmd(nc, in_maps, list(range(N_CORES)))
    except Exception:
        # transient NRT device wedge — one clean retry
        res = run_bass_kernel_spmd(nc, in_maps, list(range(N_CORES)))
    out = np.empty((1, N, OUT), np.float32)
    for c in range(N_CORES):
        out[0, c * R:(c + 1) * R, :] = res.results[c]["outT"].T
    return out



# revision 20
# speedup vs baseline: 1.1603x; 1.1603x over previous
"""Dense GAT (2-layer, 8+1 heads) on 8 Trainium2 NeuronCores — V3.

Row-parallel over destination rows i (R=512 per core). Per core:
  - adjacency arrives HOST-TRANSPOSED as adjT[j, i] in {0, 65504} fp16
    (mask applied via tensor MIN, no PE transposes needed).
  - h1|a_src from one fp16 matmul chain against host-folded
    [w1 | w1@blockdiag(att_src1)]; fp16 PSUM.
  - scaled-attention trick: softmax over j is invariant to any per-i
    factor, so E is normalized by exp(0.2*ad_i):
        e~[j,i] = min(adjT[j,i], max(eas_j * u8[i], e2as_j))
    with u8 = exp(0.8*ad_i) broadcast (one per head), eas = exp(as_j),
    e2as = exp(0.2*as_j) per-partition scalars.
    DVE chain: one tensor_scalar (4x mode) + mask-min (quad-batched).
    ACT chain (some tiles): Relu(u8*eas - e2as) + Identity(r + e2as).
    Mask-min on DVE or GPSIMD (Pool) per static schedule.
  - softmax denominators ride as a ones column in the aggregation lhsT.
  - one AllGather of [512, 66] f32 (h2+b2 packed fp16 | eas2 | e2as2).
  - L2 output normalized via per-partition ACT scale after PE transpose.
"""
import numpy as np

N = 4096
F_IN = 256
HID = 64
H1 = 8
F1 = H1 * HID
OUT = 128
N_CORES = 8
R = N // N_CORES
JT = N // 128          # 32 j-tiles
IT = R // 128          # 4 i-tiles
QT = JT // 4           # 8 quads of 4 j-tiles
NEG_ATT = 0.2
NEG_OUT = 0.01
MASKV = 65504.0        # fp16 max: adjacency "1" value; mask via min()

G = HID + 2            # bounce cols: 64 f32 words (128 f16 h2) | eas2 | e2as2

_CACHE = {}

# ---- static engine schedule knobs ----
# ACT chain quads: (h, qt) pairs routed to the scalar engine (2 ACT ops/tile)
ACT_QUADS = 18         # of 64 L1 quads -> 72 tiles on ACT


def _is_act_quad(h, qt):
    return (h * QT + qt) * 7 % 64 < ACT_QUADS


def _build():
    import concourse.bass as bass
    from concourse import bacc
    import concourse.mybir as mybir
    import concourse.tile as tile
    from concourse.masks import make_identity

    f32 = mybir.dt.float32
    f16 = mybir.dt.float16
    A = mybir.ActivationFunctionType
    Al = mybir.AluOpType

    nc = bacc.Bacc("TRN2", target_bir_lowering=False, debug=False,
                   num_devices=N_CORES)
    d_xT16 = nc.dram_tensor("xT16", [F_IN, N], f16, kind="ExternalInput")
    d_xmT = nc.dram_tensor("xmT", [F_IN, R], f32, kind="ExternalInput")
    d_adjT = nc.dram_tensor("adjT", [N, R], f16, kind="ExternalInput")
    d_rhs1 = nc.dram_tensor("rhs1", [F_IN, F1 + H1], f16, kind="ExternalInput")
    d_vdst1 = nc.dram_tensor("vdst1", [F_IN, H1], f32, kind="ExternalInput")
    d_rhs2 = nc.dram_tensor("rhs2", [F1, OUT + 2], f16, kind="ExternalInput")
    d_b1c = nc.dram_tensor("b1c", [HID, H1], f32, kind="ExternalInput")
    d_b2r = nc.dram_tensor("b2r", [1, OUT + 2], f16, kind="ExternalInput")
    d_out = nc.dram_tensor("outR", [R, OUT], f32, kind="ExternalOutput")

    with tile.TileContext(nc) as tc:
        with tc.tile_pool(name="const", bufs=1) as const, \
             tc.tile_pool(name="big", bufs=1) as big, \
             tc.tile_pool(name="work", bufs=3) as work, \
             tc.tile_pool(name="qpool", bufs=3) as qpool, \
             tc.tile_pool(name="epool", bufs=3) as epool, \
             tc.tile_pool(name="dram", bufs=1, space="DRAM") as dram, \
             tc.tile_pool(name="ps_mm", bufs=2, space="PSUM") as ps_mm, \
             tc.tile_pool(name="ps_bc", bufs=2, space="PSUM") as ps_bc, \
             tc.tile_pool(name="ps_ag", bufs=2, space="PSUM") as ps_ag, \
             tc.tile_pool(name="ps_sm", bufs=2, space="PSUM") as ps_sm:
            ident = const.tile([128, 128], f32)
            make_identity(nc, ident)
            ident16 = const.tile([128, 128], f16)
            nc.vector.tensor_copy(ident16, ident)
            ones16 = const.tile([1, 128], f16)
            nc.vector.memset(ones16, 1.0)
            ones_col16 = const.tile([128, 1], f16)
            nc.vector.memset(ones_col16, 1.0)
            rhs1_sb = const.tile([128, 2, F1 + H1], f16)
            nc.sync.dma_start(out=rhs1_sb[:, 0, :], in_=d_rhs1[0:128, :])
            nc.sync.dma_start(out=rhs1_sb[:, 1, :], in_=d_rhs1[128:256, :])
            vdst1_sb = const.tile([128, 2, H1], f32)
            nc.sync.dma_start(out=vdst1_sb[:, 0, :], in_=d_vdst1[0:128, :])
            nc.sync.dma_start(out=vdst1_sb[:, 1, :], in_=d_vdst1[128:256, :])
            rhs2_sb = const.tile([128, 4, OUT + 2], f16)
            for kt in range(4):
                nc.sync.dma_start(out=rhs2_sb[:, kt, :],
                                  in_=d_rhs2[kt * 128:(kt + 1) * 128, :])
            b1_sb = const.tile([HID, H1], f32)
            nc.sync.dma_start(out=b1_sb, in_=d_b1c[:, :])
            b2r_sb = const.tile([1, OUT + 2], f16)
            nc.sync.dma_start(out=b2r_sb, in_=d_b2r[:, :])

            # ---- big persistent arrays ----
            adjT_all = big.tile([128, JT, R], f16)       # 32 KB/part
            xT_sb = big.tile([128, 2, N], f16)           # 16 KB/part
            h1_all = big.tile([128, JT, H1, HID + 1], f16)  # 32.5 KB/part
            asrc16 = big.tile([128, JT, H1], f16)
            easrc = big.tile([128, JT, H1], f32)
            e2src = big.tile([128, JT, H1], f32)
            nege2 = big.tile([128, JT, H1], f32)
            adstT = big.tile([H1, R], f32)
            adst_rows = big.tile([1, H1, R], f32)
            adst2T = big.tile([1, R], f32)
            x2T_all = big.tile([128, 4, R], f16)
            u8bc = big.tile([128, 2, R], f16)            # 2-head pipeline
            h2g_all = big.tile([128, N_CORES, IT, G], f32)
            u8bc2 = big.tile([128, R], f16)
            rz2col = big.tile([128, IT], f32)

            nc.vector.memset(h1_all[:, :, :, HID:HID + 1], 1.0)

            # ---- input DMAs (adjT first: needed earliest at scale) ----
            for jb in range(JT):
                nc.sync.dma_start(
                    out=adjT_all[:, jb, :],
                    in_=d_adjT[jb * 128:(jb + 1) * 128, :])
            nc.sync.dma_start(out=xT_sb[:, 0, :], in_=d_xT16[0:128, :])
            nc.sync.dma_start(out=xT_sb[:, 1, :], in_=d_xT16[128:256, :])

            # ---- a_dst (own rows) -> adstT [H1, R] f32 ----
            xmT_sb = big.tile([128, 2, R], f32)
            nc.sync.dma_start(out=xmT_sb[:, 0, :], in_=d_xmT[0:128, :])
            nc.sync.dma_start(out=xmT_sb[:, 1, :], in_=d_xmT[128:256, :])
            for it in range(IT):
                ps_ad = ps_sm.tile([128, 128], f32, tag="sm")
                for kb in range(2):
                    nc.tensor.matmul(ps_ad[:, 0:H1],
                                     xmT_sb[:, kb, it * 128:(it + 1) * 128],
                                     vdst1_sb[:, kb, :],
                                     start=(kb == 0), stop=(kb == 1))
                adm = work.tile([128, H1], f32, tag="adm", bufs=2)
                nc.vector.tensor_copy(adm, ps_ad[:, 0:H1])
                ps_adT = ps_sm.tile([128, 128], f32, tag="sm")
                nc.tensor.transpose(ps_adT[0:H1, :], adm, ident)
                nc.vector.tensor_copy(adstT[:, it * 128:(it + 1) * 128],
                                      ps_adT[0:H1, :])
            for h in range(H1):
                nc.sync.dma_start(out=adst_rows[:, h, :], in_=adstT[h:h + 1, :])

            # ---- h1 | a_src per jt ----
            for jt in range(JT):
                cols = slice(jt * 128, (jt + 1) * 128)
                ps_h = ps_mm.tile([128, F1], f32, tag="h")
                ps_ast = ps_sm.tile([128, 128], f32, tag="sm")
                ps_as = ps_ast[:, 0:H1]
                for kb in range(2):
                    nc.tensor.matmul(ps_h, xT_sb[:, kb, cols],
                                     rhs1_sb[:, kb, 0:F1],
                                     start=(kb == 0), stop=(kb == 1))
                    nc.tensor.matmul(ps_as, xT_sb[:, kb, cols],
                                     rhs1_sb[:, kb, F1:F1 + H1],
                                     start=(kb == 0), stop=(kb == 1))
                # h1 [j, h, c] + as slice; copies on ACT (it has slack here)
                nc.scalar.copy(h1_all[:, jt, :, 0:HID],
                               ps_h.rearrange("p (h c) -> p h c", c=HID))
                nc.scalar.copy(asrc16[:, jt, :], ps_as)
            for g in range(4):
                gs = slice(g * 8, (g + 1) * 8)
                nc.scalar.activation(easrc[:, gs, :], asrc16[:, gs, :], A.Exp)
                nc.scalar.activation(e2src[:, gs, :], asrc16[:, gs, :], A.Exp,
                                     scale=NEG_ATT)
                nc.scalar.activation(nege2[:, gs, :], e2src[:, gs, :],
                                     A.Identity, scale=-1.0)

            # ---- layer-1 attention, head-pipelined ----
            def _pre_head(h):
                u8row = work.tile([1, R], f16, tag="u8row", bufs=2,
                                  name=f"u8r{h}")
                nc.scalar.activation(u8row, adst_rows[:, h, :], A.Exp,
                                     scale=1.0 - NEG_ATT)
                ps_u8 = ps_bc.tile([128, R], f32, tag="bc", name=f"psu8{h}")
                nc.tensor.matmul(ps_u8, ones16, u8row, start=True, stop=True)
                nc.vector.tensor_copy(u8bc[:, h % 2, :], ps_u8)

            _pre_head(0)
            for h in range(H1):
                if h + 1 < H1:
                    _pre_head(h + 1)
                u8 = u8bc[:, h % 2, :]
                ps_agg = ps_ag.tile([HID + 1, R], f32, tag="agg")
                for qt in range(QT):
                    q4 = qpool.tile([128, 4, R], f16, tag="q")
                    for k in range(4):
                        jt = qt * 4 + k
                        if _is_act_quad(h, qt):
                            r = work.tile([128, R], f16, tag="ract", bufs=3)
                            nc.scalar.activation(
                                r, u8, A.Relu,
                                bias=nege2[:, jt, h:h + 1],
                                scale=easrc[:, jt, h:h + 1])
                            nc.scalar.activation(
                                q4[:, k, :], r, A.Identity,
                                bias=e2src[:, jt, h:h + 1])
                        else:
                            nc.vector.tensor_scalar(
                                q4[:, k, :], u8,
                                easrc[:, jt, h:h + 1],
                                e2src[:, jt, h:h + 1],
                                op0=Al.mult, op1=Al.max)
                    e4 = epool.tile([128, 4, R], f16, tag="e")
                    nc.vector.tensor_tensor(
                        e4.rearrange("p a b -> p (a b)"),
                        q4.rearrange("p a b -> p (a b)"),
                        adjT_all[:, qt * 4:(qt + 1) * 4, :].rearrange(
                            "p a b -> p (a b)"),
                        op=Al.min)
                    for k in range(4):
                        jt = qt * 4 + k
                        nc.tensor.matmul(ps_agg, h1_all[:, jt, h, :],
                                         e4[:, k, :],
                                         start=(jt == 0), stop=(jt == JT - 1))
                rz = work.tile([1, R], f16, tag="rz", bufs=2)
                with nc.allow_low_precision(reason="1/z in fp16: 1e-3 rel ok"):
                    nc.vector.reciprocal(rz, ps_agg[HID:HID + 1, :])
                ps_rzb = ps_bc.tile([128, R], f32, tag="bc")
                nc.tensor.matmul(ps_rzb[0:HID, :], ones16[:, 0:HID], rz,
                                 start=True, stop=True)
                rzb_sb = work.tile([HID, R], f16, tag="rzb", bufs=2)
                nc.scalar.copy(rzb_sb, ps_rzb[0:HID, :])
                y_h = work.tile([HID, R], f16, tag="yh", bufs=2)
                nc.vector.tensor_mul(y_h, ps_agg[0:HID, :], rzb_sb)
                po = (h % 2) * HID
                nc.scalar.activation(
                    x2T_all[po:po + HID, h // 2, :], y_h, A.Prelu,
                    bias=b1_sb[:, h:h + 1], alpha=NEG_OUT)

            # ---- layer 2: h2 per it, bounce, AllGather ----
            bounce_in = dram.tile([R, G], f32, name="bin")
            bounce_out = dram.tile([N_CORES, R, G], f32,
                                   addr_space="Shared", name="bout")
            for it in range(IT):
                ps_h2t = ps_mm.tile([128, R], f32, tag="h")
                ps_h2 = ps_h2t[:, 0:OUT + 2]
                for kt in range(4):
                    nc.tensor.matmul(
                        ps_h2,
                        x2T_all[:, kt, it * 128:(it + 1) * 128],
                        rhs2_sb[:, kt, :],
                        start=(kt == 0), stop=False)
                nc.tensor.matmul(ps_h2, ones16, b2r_sb,
                                 start=False, stop=True)
                h2m = work.tile([128, G], f32, tag="h2m", bufs=2)
                nc.vector.tensor_copy(h2m[:, 0:HID].bitcast(f16),
                                      ps_h2[:, 0:OUT])
                nc.scalar.activation(h2m[:, HID:HID + 1],
                                     ps_h2[:, OUT:OUT + 1], A.Exp)
                nc.scalar.activation(h2m[:, HID + 1:HID + 2],
                                     ps_h2[:, OUT:OUT + 1], A.Exp,
                                     scale=NEG_ATT)
                nc.sync.dma_start(
                    out=bounce_in[it * 128:(it + 1) * 128, :], in_=h2m)
                ad2m = work.tile([128, 1], f32, tag="ad2m", bufs=2)
                nc.scalar.copy(ad2m, ps_h2[:, OUT + 1:OUT + 2])
                ps_adT2 = ps_sm.tile([1, 128], f32, tag="sm")
                nc.tensor.transpose(ps_adT2, ad2m, ident)
                nc.vector.tensor_copy(adst2T[:, it * 128:(it + 1) * 128],
                                      ps_adT2)
            nc.gpsimd.collective_compute(
                "AllGather",
                bass.mybir.AluOpType.bypass,
                replica_groups=[list(range(N_CORES))],
                ins=[bounce_in.opt()],
                outs=[bounce_out.opt()],
            )
            for c8 in range(N_CORES):
                nc.sync.dma_start(
                    out=h2g_all[:, c8, :, :],
                    in_=bounce_out[c8].rearrange("(r1 p) g -> p r1 g", p=128))

            # ---- layer-2 attention ----
            u8row2 = work.tile([1, R], f16, tag="u8row", bufs=2)
            nc.scalar.activation(u8row2, adst2T, A.Exp, scale=1.0 - NEG_ATT)
            ps_u82 = ps_bc.tile([128, R], f32, tag="bc")
            nc.tensor.matmul(ps_u82, ones16, u8row2, start=True, stop=True)
            nc.vector.tensor_copy(u8bc2, ps_u82)

            # reuse L1 pools: o2 in ps_mm "h" shape, z2 rides an "agg" buffer
            ps_o2 = ps_mm.tile([128, R], f32, tag="h")
            ps_z2t = ps_ag.tile([HID + 1, R], f32, tag="agg")
            ps_z2 = ps_z2t[HID:HID + 1, :]
            for qt in range(QT):
                q4 = qpool.tile([128, 4, R], f16, tag="q")
                for k in range(4):
                    jt = qt * 4 + k
                    c8, r1 = jt // IT, jt % IT
                    nc.vector.tensor_scalar(
                        q4[:, k, :], u8bc2,
                        h2g_all[:, c8, r1, HID:HID + 1],
                        h2g_all[:, c8, r1, HID + 1:HID + 2],
                        op0=Al.mult, op1=Al.max)
                e4 = epool.tile([128, 4, R], f16, tag="e")
                nc.vector.tensor_tensor(
                    e4.rearrange("p a b -> p (a b)"),
                    q4.rearrange("p a b -> p (a b)"),
                    adjT_all[:, qt * 4:(qt + 1) * 4, :].rearrange(
                        "p a b -> p (a b)"),
                    op=Al.min)
                for k in range(4):
                    jt = qt * 4 + k
                    c8, r1 = jt // IT, jt % IT
                    nc.tensor.matmul(
                        ps_o2, h2g_all[:, c8, r1, 0:HID].bitcast(f16),
                        e4[:, k, :],
                        start=(jt == 0), stop=(jt == JT - 1))
                    nc.tensor.matmul(
                        ps_z2, ones_col16, e4[:, k, :],
                        start=(jt == 0), stop=(jt == JT - 1))
            # per-it: transpose z2 chunk -> recip col; transpose o2 -> prelu
            o2sb = work.tile([128, R], f32, tag="o2sb", bufs=1)
            for it in range(IT):
                nc.scalar.copy(o2sb[:, it * 128:(it + 1) * 128],
                               ps_o2[:, it * 128:(it + 1) * 128])
            z2sb = work.tile([1, R], f16, tag="z2sb", bufs=1)
            nc.vector.tensor_copy(z2sb, ps_z2)
            del ps_z2t
            outT_sb = work.tile([128, IT, OUT], f32, tag="outT", bufs=1)
            for it in range(IT):
                isl = slice(it * 128, (it + 1) * 128)
                ps_zTt = ps_sm.tile([128, 128], f32, tag="sm")
                ps_zT = ps_zTt.bitcast(f16)[:, 0:1]
                nc.tensor.transpose(ps_zT, z2sb[:, isl], ident16[0:1, 0:1])
                with nc.allow_low_precision(reason="1/z2 col fp16 src ok"):
                    nc.vector.reciprocal(rz2col[:, it:it + 1], ps_zT)
                ps_oT = ps_sm.tile([128, 128], f32, tag="sm")
                nc.tensor.transpose(ps_oT, o2sb[:, isl], ident)
                nc.scalar.activation(outT_sb[:, it, :], ps_oT, A.Prelu,
                                     scale=rz2col[:, it:it + 1], alpha=NEG_OUT)
            nc.sync.dma_start(
                out=d_out.rearrange("(i p) c -> p i c", p=128), in_=outT_sb)

    nc.finalize()
    return nc


def _prep_host(x, adj, w1, att_src1, att_dst1, b1, w2, att_src2, att_dst2, b2):
    x = np.asarray(x, np.float32).reshape(N, F_IN)
    adj = np.asarray(adj, np.float32).reshape(N, N)
    w1 = np.asarray(w1, np.float32)
    w2 = np.asarray(w2, np.float32)
    att_src1 = np.asarray(att_src1, np.float32)
    att_dst1 = np.asarray(att_dst1, np.float32)
    att_src2 = np.asarray(att_src2, np.float32)
    att_dst2 = np.asarray(att_dst2, np.float32)
    b1 = np.asarray(b1, np.float32)
    b2 = np.asarray(b2, np.float32)

    xT = np.ascontiguousarray(x.T)
    xT16 = xT.astype(np.float16)
    adjm = (adj * MASKV).astype(np.float16)
    v_src1 = np.empty((F_IN, H1), np.float32)
    v_dst1 = np.empty((F_IN, H1), np.float32)
    for h in range(H1):
        blk = w1[:, h * HID:(h + 1) * HID]
        v_src1[:, h] = blk @ att_src1[h]
        v_dst1[:, h] = blk @ att_dst1[h]
    rhs1 = np.ascontiguousarray(
        np.concatenate([w1, v_src1], axis=1)).astype(np.float16)
    v_src2 = (w2 @ att_src2[0])[:, None]
    v_dst2 = (w2 @ att_dst2[0])[:, None]
    rhs2 = np.ascontiguousarray(
        np.concatenate([w2, v_src2, v_dst2], axis=1)).astype(np.float16)
    b1c = np.ascontiguousarray(b1.reshape(H1, HID).T)
    b2r = np.zeros((1, OUT + 2), np.float16)
    b2r[0, 0:OUT] = b2

    in_maps = []
    for c in range(N_CORES):
        rows = slice(c * R, (c + 1) * R)
        in_maps.append({
            "xT16": xT16,
            "xmT": np.ascontiguousarray(xT[:, rows]),
            "adjT": np.ascontiguousarray(adjm[rows, :].T),
            "rhs1": rhs1,
            "vdst1": v_dst1,
            "rhs2": rhs2,
            "b1c": b1c,
            "b2r": b2r,
        })
    return in_maps


def kernel(**inputs) -> np.ndarray:
    from concourse.bass_utils import run_bass_kernel_spmd

    if "nc" not in _CACHE:
        _CACHE["nc"] = _build()
    nc = _CACHE["nc"]
    in_maps = _prep_host(**inputs)
    try:
        res = run_bass_kernel_spmd(nc, in_maps, list(range(N_CORES)))
    except Exception:
        # transient NRT device wedge — one clean retry
        res = run_bass_kernel_spmd(nc, in_maps, list(range(N_CORES)))
    out = np.empty((1, N, OUT), np.float32)
    for c in range(N_CORES):
        out[0, c * R:(c + 1) * R, :] = res.results[c]["outR"]
    return out


# revision 21
# speedup vs baseline: 1.4229x; 1.2264x over previous
"""Dense GAT (2-layer, 8+1 heads) on 8 Trainium2 NeuronCores — V3.

Row-parallel over destination rows i (R=512 per core). Per core:
  - adjacency arrives HOST-TRANSPOSED as adjT[j, i] in {0, 65504} fp16
    (mask applied via tensor MIN, no PE transposes needed).
  - h1|a_src from one fp16 matmul chain against host-folded
    [w1 | w1@blockdiag(att_src1)]; fp16 PSUM.
  - scaled-attention trick: softmax over j is invariant to any per-i
    factor, so E is normalized by exp(0.2*ad_i):
        e~[j,i] = min(adjT[j,i], max(eas_j * u8[i], e2as_j))
    with u8 = exp(0.8*ad_i) broadcast (one per head), eas = exp(as_j),
    e2as = exp(0.2*as_j) per-partition scalars.
    DVE chain: one tensor_scalar (4x mode) + mask-min (quad-batched).
    ACT chain (some tiles): Relu(u8*eas - e2as) + Identity(r + e2as).
    Mask-min on DVE or GPSIMD (Pool) per static schedule.
  - softmax denominators ride as a ones column in the aggregation lhsT.
  - one AllGather of [512, 66] f32 (h2+b2 packed fp16 | eas2 | e2as2).
  - L2 output normalized via per-partition ACT scale after PE transpose.
"""
import numpy as np

N = 4096
F_IN = 256
HID = 64
H1 = 8
F1 = H1 * HID
OUT = 128
N_CORES = 8
R = N // N_CORES
JT = N // 128          # 32 j-tiles
IT = R // 128          # 4 i-tiles
QT = JT // 4           # 8 quads of 4 j-tiles
NEG_ATT = 0.2
NEG_OUT = 0.01
MASKV = 65504.0        # fp16 max: adjacency "1" value; mask via min()

G = HID + 2            # bounce cols: 64 f32 words (128 f16 h2) | eas2 | e2as2

_CACHE = {}

# ---- static engine schedule knobs ----
# ACT chain quads: (h, qt) pairs routed to the scalar engine (2 ACT ops/tile)
ACT_QUADS = 18         # of 64 L1 quads -> 72 tiles on ACT


def _is_act_quad(h, qt):
    return (h * QT + qt) * 7 % 64 < ACT_QUADS


def _build():
    import concourse.bass as bass
    from concourse import bacc
    import concourse.mybir as mybir
    import concourse.tile as tile
    from concourse.masks import make_identity

    f32 = mybir.dt.float32
    f16 = mybir.dt.float16
    A = mybir.ActivationFunctionType
    Al = mybir.AluOpType

    nc = bacc.Bacc("TRN2", target_bir_lowering=False, debug=False,
                   num_devices=N_CORES)
    d_xT16 = nc.dram_tensor("xT16", [F_IN, N], f16, kind="ExternalInput")
    d_xmT = nc.dram_tensor("xmT", [F_IN, R], f32, kind="ExternalInput")
    d_adjT = nc.dram_tensor("adjT", [N, R], f16, kind="ExternalInput")
    d_rhs1 = nc.dram_tensor("rhs1", [F_IN, F1 + H1], f16, kind="ExternalInput")
    d_vdst1 = nc.dram_tensor("vdst1", [F_IN, H1], f32, kind="ExternalInput")
    d_rhs2 = nc.dram_tensor("rhs2", [F1, OUT + 2], f16, kind="ExternalInput")
    d_b1c = nc.dram_tensor("b1c", [HID, H1], f32, kind="ExternalInput")
    d_b2r = nc.dram_tensor("b2r", [1, OUT + 2], f16, kind="ExternalInput")
    d_out = nc.dram_tensor("outR", [R, OUT], f32, kind="ExternalOutput")

    with tile.TileContext(nc) as tc:
        with tc.tile_pool(name="const", bufs=1) as const, \
             tc.tile_pool(name="big", bufs=1) as big, \
             tc.tile_pool(name="work", bufs=3) as work, \
             tc.tile_pool(name="qpool", bufs=3) as qpool, \
             tc.tile_pool(name="epool", bufs=3) as epool, \
             tc.tile_pool(name="dram", bufs=1, space="DRAM") as dram, \
             tc.tile_pool(name="ps_mm", bufs=2, space="PSUM") as ps_mm, \
             tc.tile_pool(name="ps_bc", bufs=2, space="PSUM") as ps_bc, \
             tc.tile_pool(name="ps_ag", bufs=2, space="PSUM") as ps_ag, \
             tc.tile_pool(name="ps_sm", bufs=2, space="PSUM") as ps_sm:
            ident = const.tile([128, 128], f32)
            make_identity(nc, ident)
            ident16 = const.tile([128, 128], f16)
            nc.vector.tensor_copy(ident16, ident)
            ones16 = const.tile([1, 128], f16)
            nc.vector.memset(ones16, 1.0)
            ones_col16 = const.tile([128, 1], f16)
            nc.vector.memset(ones_col16, 1.0)
            rhs1_sb = const.tile([128, 2, F1 + H1], f16)
            nc.sync.dma_start(out=rhs1_sb[:, 0, :], in_=d_rhs1[0:128, :])
            nc.sync.dma_start(out=rhs1_sb[:, 1, :], in_=d_rhs1[128:256, :])
            vdst1_sb = const.tile([128, 2, H1], f32)
            nc.sync.dma_start(out=vdst1_sb[:, 0, :], in_=d_vdst1[0:128, :])
            nc.sync.dma_start(out=vdst1_sb[:, 1, :], in_=d_vdst1[128:256, :])
            rhs2_sb = const.tile([128, 4, OUT + 2], f16)
            for kt in range(4):
                nc.sync.dma_start(out=rhs2_sb[:, kt, :],
                                  in_=d_rhs2[kt * 128:(kt + 1) * 128, :])
            b1_sb = const.tile([HID, H1], f32)
            nc.sync.dma_start(out=b1_sb, in_=d_b1c[:, :])
            b2r_sb = const.tile([1, OUT + 2], f16)
            nc.sync.dma_start(out=b2r_sb, in_=d_b2r[:, :])

            # ---- big persistent arrays ----
            adjT_all = big.tile([128, JT, R], f16)       # 32 KB/part
            xT_sb = big.tile([128, 2, N], f16)           # 16 KB/part
            h1_all = big.tile([128, JT, H1, HID + 1], f16)  # 32.5 KB/part
            asrc16 = big.tile([128, JT, H1], f16)
            easrc = big.tile([128, JT, H1], f32)
            e2src = big.tile([128, JT, H1], f32)
            nege2 = big.tile([128, JT, H1], f32)
            adstT = big.tile([H1, R], f32)
            adst_rows = big.tile([1, H1, R], f32)
            adst2T = big.tile([1, R], f32)
            x2T_all = big.tile([128, 4, R], f16)
            u8bc = big.tile([128, 2, R], f16)            # 2-head pipeline
            h2g_all = big.tile([128, N_CORES, IT, G], f32)
            u8bc2 = big.tile([128, R], f16)
            rz2col = big.tile([128, IT], f32)

            nc.vector.memset(h1_all[:, :, :, HID:HID + 1], 1.0)

            # ---- input DMAs: xT first (h1 needs it), adjT streams after ----
            nc.sync.dma_start(out=xT_sb[:, 0, :], in_=d_xT16[0:128, :])
            nc.sync.dma_start(out=xT_sb[:, 1, :], in_=d_xT16[128:256, :])
            for jb in range(JT):
                nc.sync.dma_start(
                    out=adjT_all[:, jb, :],
                    in_=d_adjT[jb * 128:(jb + 1) * 128, :])

            # ---- a_dst (own rows) -> adstT [H1, R] f32 ----
            xmT_sb = big.tile([128, 2, R], f32)
            nc.sync.dma_start(out=xmT_sb[:, 0, :], in_=d_xmT[0:128, :])
            nc.sync.dma_start(out=xmT_sb[:, 1, :], in_=d_xmT[128:256, :])
            for it in range(IT):
                ps_ad = ps_sm.tile([128, 128], f32, tag="sm")
                for kb in range(2):
                    nc.tensor.matmul(ps_ad[:, 0:H1],
                                     xmT_sb[:, kb, it * 128:(it + 1) * 128],
                                     vdst1_sb[:, kb, :],
                                     start=(kb == 0), stop=(kb == 1))
                adm = work.tile([128, H1], f32, tag="adm", bufs=2)
                nc.vector.tensor_copy(adm, ps_ad[:, 0:H1])
                ps_adT = ps_sm.tile([128, 128], f32, tag="sm")
                nc.tensor.transpose(ps_adT[0:H1, :], adm, ident)
                nc.vector.tensor_copy(adstT[:, it * 128:(it + 1) * 128],
                                      ps_adT[0:H1, :])
            for h in range(H1):
                nc.sync.dma_start(out=adst_rows[:, h, :], in_=adstT[h:h + 1, :])

            # ---- h1 | a_src per jt ----
            for jt in range(JT):
                cols = slice(jt * 128, (jt + 1) * 128)
                ps_h = ps_mm.tile([128, F1], f32, tag="h")
                ps_ast = ps_sm.tile([128, 128], f32, tag="sm")
                ps_as = ps_ast[:, 0:H1]
                for kb in range(2):
                    nc.tensor.matmul(ps_h, xT_sb[:, kb, cols],
                                     rhs1_sb[:, kb, 0:F1],
                                     start=(kb == 0), stop=(kb == 1))
                    nc.tensor.matmul(ps_as, xT_sb[:, kb, cols],
                                     rhs1_sb[:, kb, F1:F1 + H1],
                                     start=(kb == 0), stop=(kb == 1))
                # h1 [j, h, c] + as slice; split copies across DVE/ACT
                if jt % 2 == 0:
                    nc.vector.tensor_copy(
                        h1_all[:, jt, :, 0:HID],
                        ps_h.rearrange("p (h c) -> p h c", c=HID))
                else:
                    nc.scalar.copy(
                        h1_all[:, jt, :, 0:HID],
                        ps_h.rearrange("p (h c) -> p h c", c=HID))
                nc.scalar.copy(asrc16[:, jt, :], ps_as)
                if jt % 8 == 7:
                    gs = slice(jt - 7, jt + 1)
                    nc.scalar.activation(easrc[:, gs, :], asrc16[:, gs, :],
                                         A.Exp)
                    nc.scalar.activation(e2src[:, gs, :], asrc16[:, gs, :],
                                         A.Exp, scale=NEG_ATT)
                    nc.scalar.activation(nege2[:, gs, :], e2src[:, gs, :],
                                         A.Identity, scale=-1.0)

            # ---- layer-1 attention, head-pipelined ----
            def _pre_head(h):
                u8row = work.tile([1, R], f16, tag="u8row", bufs=2,
                                  name=f"u8r{h}")
                nc.scalar.activation(u8row, adst_rows[:, h, :], A.Exp,
                                     scale=1.0 - NEG_ATT)
                ps_u8 = ps_bc.tile([128, R], f32, tag="bc", name=f"psu8{h}")
                nc.tensor.matmul(ps_u8, ones16, u8row, start=True, stop=True)
                nc.vector.tensor_copy(u8bc[:, h % 2, :], ps_u8)

            _pre_head(0)
            for h in range(H1):
                if h + 1 < H1:
                    _pre_head(h + 1)
                u8 = u8bc[:, h % 2, :]
                ps_agg = ps_ag.tile([HID + 1, R], f32, tag="agg")
                act_qts = [qt for qt in range(QT) if _is_act_quad(h, qt)]
                dve_qts = [qt for qt in range(QT) if not _is_act_quad(h, qt)]
                # ACT-chain q tiles first so the scalar engine runs ahead
                act_q4 = {}
                for qt in act_qts:
                    q4 = qpool.tile([128, 4, R], f16, tag="qa", bufs=3,
                                    name=f"qa{h}_{qt}")
                    for k in range(4):
                        jt = qt * 4 + k
                        r = work.tile([128, R], f16, tag="ract", bufs=3)
                        nc.scalar.activation(
                            r, u8, A.Relu,
                            bias=nege2[:, jt, h:h + 1],
                            scale=easrc[:, jt, h:h + 1])
                        nc.scalar.activation(
                            q4[:, k, :], r, A.Identity,
                            bias=e2src[:, jt, h:h + 1])
                    act_q4[qt] = q4
                n_mm = 0
                for qt in dve_qts + act_qts:
                    if qt in act_q4:
                        q4 = act_q4[qt]
                    else:
                        q4 = qpool.tile([128, 4, R], f16, tag="q")
                        for k in range(4):
                            jt = qt * 4 + k
                            nc.vector.tensor_scalar(
                                q4[:, k, :], u8,
                                easrc[:, jt, h:h + 1],
                                e2src[:, jt, h:h + 1],
                                op0=Al.mult, op1=Al.max)
                    e4 = epool.tile([128, 4, R], f16, tag="e")
                    nc.vector.tensor_tensor(
                        e4.rearrange("p a b -> p (a b)"),
                        q4.rearrange("p a b -> p (a b)"),
                        adjT_all[:, qt * 4:(qt + 1) * 4, :].rearrange(
                            "p a b -> p (a b)"),
                        op=Al.min)
                    for k in range(4):
                        jt = qt * 4 + k
                        nc.tensor.matmul(ps_agg, h1_all[:, jt, h, :],
                                         e4[:, k, :],
                                         start=(n_mm == 0),
                                         stop=(n_mm == JT - 1))
                        n_mm += 1
                rz = work.tile([1, R], f16, tag="rz", bufs=2)
                with nc.allow_low_precision(reason="1/z in fp16: 1e-3 rel ok"):
                    nc.vector.reciprocal(rz, ps_agg[HID:HID + 1, :])
                ps_rzb = ps_bc.tile([128, R], f32, tag="bc")
                nc.tensor.matmul(ps_rzb[0:HID, :], ones16[:, 0:HID], rz,
                                 start=True, stop=True)
                rzb_sb = work.tile([HID, R], f16, tag="rzb", bufs=2)
                nc.scalar.copy(rzb_sb, ps_rzb[0:HID, :])
                y_h = work.tile([HID, R], f16, tag="yh", bufs=2)
                nc.vector.tensor_mul(y_h, ps_agg[0:HID, :], rzb_sb)
                po = (h % 2) * HID
                nc.scalar.activation(
                    x2T_all[po:po + HID, h // 2, :], y_h, A.Prelu,
                    bias=b1_sb[:, h:h + 1], alpha=NEG_OUT)

            # ---- layer 2: h2 per it, bounce, AllGather ----
            bounce_in = dram.tile([R, G], f32, name="bin")
            bounce_out = dram.tile([N_CORES, R, G], f32,
                                   addr_space="Shared", name="bout")
            for it in range(IT):
                ps_h2t = ps_mm.tile([128, R], f32, tag="h")
                ps_h2 = ps_h2t[:, 0:OUT + 2]
                for kt in range(4):
                    nc.tensor.matmul(
                        ps_h2,
                        x2T_all[:, kt, it * 128:(it + 1) * 128],
                        rhs2_sb[:, kt, :],
                        start=(kt == 0), stop=False)
                nc.tensor.matmul(ps_h2, ones16, b2r_sb,
                                 start=False, stop=True)
                h2m = work.tile([128, G], f32, tag="h2m", bufs=2)
                nc.vector.tensor_copy(h2m[:, 0:HID].bitcast(f16),
                                      ps_h2[:, 0:OUT])
                nc.scalar.activation(h2m[:, HID:HID + 1],
                                     ps_h2[:, OUT:OUT + 1], A.Exp)
                nc.scalar.activation(h2m[:, HID + 1:HID + 2],
                                     ps_h2[:, OUT:OUT + 1], A.Exp,
                                     scale=NEG_ATT)
                nc.sync.dma_start(
                    out=bounce_in[it * 128:(it + 1) * 128, :], in_=h2m)
                ad2m = work.tile([128, 1], f32, tag="ad2m", bufs=2)
                nc.scalar.copy(ad2m, ps_h2[:, OUT + 1:OUT + 2])
                ps_adT2 = ps_sm.tile([1, 128], f32, tag="sm")
                nc.tensor.transpose(ps_adT2, ad2m, ident)
                nc.vector.tensor_copy(adst2T[:, it * 128:(it + 1) * 128],
                                      ps_adT2)
            nc.gpsimd.collective_compute(
                "AllGather",
                bass.mybir.AluOpType.bypass,
                replica_groups=[list(range(N_CORES))],
                ins=[bounce_in.opt()],
                outs=[bounce_out.opt()],
            )
            for c8 in range(N_CORES):
                nc.sync.dma_start(
                    out=h2g_all[:, c8, :, :],
                    in_=bounce_out[c8].rearrange("(r1 p) g -> p r1 g", p=128))

            # ---- layer-2 attention ----
            u8row2 = work.tile([1, R], f16, tag="u8row", bufs=2)
            nc.scalar.activation(u8row2, adst2T, A.Exp, scale=1.0 - NEG_ATT)
            ps_u82 = ps_bc.tile([128, R], f32, tag="bc")
            nc.tensor.matmul(ps_u82, ones16, u8row2, start=True, stop=True)
            nc.vector.tensor_copy(u8bc2, ps_u82)

            # reuse L1 pools: o2 in ps_mm "h" shape, z2 rides an "agg" buffer
            ps_o2 = ps_mm.tile([128, R], f32, tag="h")
            ps_z2t = ps_ag.tile([HID + 1, R], f32, tag="agg")
            ps_z2 = ps_z2t[HID:HID + 1, :]
            for qt in range(QT):
                q4 = qpool.tile([128, 4, R], f16, tag="q")
                for k in range(4):
                    jt = qt * 4 + k
                    c8, r1 = jt // IT, jt % IT
                    nc.vector.tensor_scalar(
                        q4[:, k, :], u8bc2,
                        h2g_all[:, c8, r1, HID:HID + 1],
                        h2g_all[:, c8, r1, HID + 1:HID + 2],
                        op0=Al.mult, op1=Al.max)
                e4 = epool.tile([128, 4, R], f16, tag="e")
                nc.vector.tensor_tensor(
                    e4.rearrange("p a b -> p (a b)"),
                    q4.rearrange("p a b -> p (a b)"),
                    adjT_all[:, qt * 4:(qt + 1) * 4, :].rearrange(
                        "p a b -> p (a b)"),
                    op=Al.min)
                for k in range(4):
                    jt = qt * 4 + k
                    c8, r1 = jt // IT, jt % IT
                    nc.tensor.matmul(
                        ps_o2, h2g_all[:, c8, r1, 0:HID].bitcast(f16),
                        e4[:, k, :],
                        start=(jt == 0), stop=(jt == JT - 1))
                    nc.tensor.matmul(
                        ps_z2, ones_col16, e4[:, k, :],
                        start=(jt == 0), stop=(jt == JT - 1))
            # per-it: transpose z2 chunk -> recip col; transpose o2 -> prelu
            o2sb = work.tile([128, R], f32, tag="o2sb", bufs=1)
            for it in range(IT):
                nc.scalar.copy(o2sb[:, it * 128:(it + 1) * 128],
                               ps_o2[:, it * 128:(it + 1) * 128])
            z2sb = work.tile([1, R], f16, tag="z2sb", bufs=1)
            nc.vector.tensor_copy(z2sb, ps_z2)
            del ps_z2t
            outT_sb = work.tile([128, IT, OUT], f32, tag="outT", bufs=1)
            for it in range(IT):
                isl = slice(it * 128, (it + 1) * 128)
                ps_zTt = ps_sm.tile([128, 128], f32, tag="sm")
                ps_zT = ps_zTt.bitcast(f16)[:, 0:1]
                nc.tensor.transpose(ps_zT, z2sb[:, isl], ident16[0:1, 0:1])
                with nc.allow_low_precision(reason="1/z2 col fp16 src ok"):
                    nc.vector.reciprocal(rz2col[:, it:it + 1], ps_zT)
                ps_oT = ps_sm.tile([128, 128], f32, tag="sm")
                nc.tensor.transpose(ps_oT, o2sb[:, isl], ident)
                nc.scalar.activation(outT_sb[:, it, :], ps_oT, A.Prelu,
                                     scale=rz2col[:, it:it + 1], alpha=NEG_OUT)
            nc.sync.dma_start(
                out=d_out.rearrange("(i p) c -> p i c", p=128), in_=outT_sb)

    nc.finalize()
    return nc


def _prep_host(x, adj, w1, att_src1, att_dst1, b1, w2, att_src2, att_dst2, b2):
    x = np.asarray(x, np.float32).reshape(N, F_IN)
    adj = np.asarray(adj, np.float32).reshape(N, N)
    w1 = np.asarray(w1, np.float32)
    w2 = np.asarray(w2, np.float32)
    att_src1 = np.asarray(att_src1, np.float32)
    att_dst1 = np.asarray(att_dst1, np.float32)
    att_src2 = np.asarray(att_src2, np.float32)
    att_dst2 = np.asarray(att_dst2, np.float32)
    b1 = np.asarray(b1, np.float32)
    b2 = np.asarray(b2, np.float32)

    xT = np.ascontiguousarray(x.T)
    xT16 = xT.astype(np.float16)
    adjm = (adj * MASKV).astype(np.float16)
    v_src1 = np.empty((F_IN, H1), np.float32)
    v_dst1 = np.empty((F_IN, H1), np.float32)
    for h in range(H1):
        blk = w1[:, h * HID:(h + 1) * HID]
        v_src1[:, h] = blk @ att_src1[h]
        v_dst1[:, h] = blk @ att_dst1[h]
    rhs1 = np.ascontiguousarray(
        np.concatenate([w1, v_src1], axis=1)).astype(np.float16)
    v_src2 = (w2 @ att_src2[0])[:, None]
    v_dst2 = (w2 @ att_dst2[0])[:, None]
    rhs2 = np.ascontiguousarray(
        np.concatenate([w2, v_src2, v_dst2], axis=1)).astype(np.float16)
    b1c = np.ascontiguousarray(b1.reshape(H1, HID).T)
    b2r = np.zeros((1, OUT + 2), np.float16)
    b2r[0, 0:OUT] = b2

    in_maps = []
    for c in range(N_CORES):
        rows = slice(c * R, (c + 1) * R)
        in_maps.append({
            "xT16": xT16,
            "xmT": np.ascontiguousarray(xT[:, rows]),
            "adjT": np.ascontiguousarray(adjm[rows, :].T),
            "rhs1": rhs1,
            "vdst1": v_dst1,
            "rhs2": rhs2,
            "b1c": b1c,
            "b2r": b2r,
        })
    return in_maps


def kernel(**inputs) -> np.ndarray:
    from concourse.bass_utils import run_bass_kernel_spmd

    if "nc" not in _CACHE:
        _CACHE["nc"] = _build()
    nc = _CACHE["nc"]
    in_maps = _prep_host(**inputs)
    try:
        res = run_bass_kernel_spmd(nc, in_maps, list(range(N_CORES)))
    except Exception:
        # transient NRT device wedge — one clean retry
        res = run_bass_kernel_spmd(nc, in_maps, list(range(N_CORES)))
    out = np.empty((1, N, OUT), np.float32)
    for c in range(N_CORES):
        out[0, c * R:(c + 1) * R, :] = res.results[c]["outR"]
    return out


# revision 24
# speedup vs baseline: 1.4848x; 1.0435x over previous
"""Dense GAT (2-layer, 8+1 heads) on 8 Trainium2 NeuronCores — V3.

Row-parallel over destination rows i (R=512 per core). Per core:
  - adjacency arrives HOST-TRANSPOSED as adjT[j, i] in {0, 65504} fp16
    (mask applied via tensor MIN, no PE transposes needed).
  - h1|a_src from one fp16 matmul chain against host-folded
    [w1 | w1@blockdiag(att_src1)]; fp16 PSUM.
  - scaled-attention trick: softmax over j is invariant to any per-i
    factor, so E is normalized by exp(0.2*ad_i):
        e~[j,i] = min(adjT[j,i], max(eas_j * u8[i], e2as_j))
    with u8 = exp(0.8*ad_i) broadcast (one per head), eas = exp(as_j),
    e2as = exp(0.2*as_j) per-partition scalars.
    DVE chain: one tensor_scalar (4x mode) + mask-min (quad-batched).
    ACT chain (some tiles): Relu(u8*eas - e2as) + Identity(r + e2as).
    Mask-min on DVE or GPSIMD (Pool) per static schedule.
  - softmax denominators ride as a ones column in the aggregation lhsT.
  - one AllGather of [512, 66] f32 (h2+b2 packed fp16 | eas2 | e2as2).
  - L2 output normalized via per-partition ACT scale after PE transpose.
"""
import numpy as np

N = 4096
F_IN = 256
HID = 64
H1 = 8
F1 = H1 * HID
OUT = 128
N_CORES = 8
R = N // N_CORES
JT = N // 128          # 32 j-tiles
IT = R // 128          # 4 i-tiles
QT = JT // 4           # 8 quads of 4 j-tiles
NEG_ATT = 0.2
NEG_OUT = 0.01
MASKV = 65504.0        # fp16 max: adjacency "1" value; mask via min()

G = HID + 2            # bounce cols: 64 f32 words (128 f16 h2) | eas2 | e2as2

_CACHE = {}

# ---- static engine schedule knobs ----
# ACT chain quads: (h, qt) pairs routed to the scalar engine (2 ACT ops/tile)
ACT_QUADS = 18         # of 64 L1 quads -> 72 tiles on ACT


def _is_act_quad(h, qt):
    return (h * QT + qt) * 7 % 64 < ACT_QUADS


def _build():
    import concourse.bass as bass
    from concourse import bacc
    import concourse.mybir as mybir
    import concourse.tile as tile
    from concourse.masks import make_identity

    f32 = mybir.dt.float32
    f16 = mybir.dt.float16
    A = mybir.ActivationFunctionType
    Al = mybir.AluOpType

    nc = bacc.Bacc("TRN2", target_bir_lowering=False, debug=False,
                   num_devices=N_CORES)
    d_xT16 = nc.dram_tensor("xT16", [F_IN, N], f16, kind="ExternalInput")
    d_xmT = nc.dram_tensor("xmT", [F_IN, R], f16, kind="ExternalInput")
    d_adjT = nc.dram_tensor("adjT", [N, R], f16, kind="ExternalInput")
    d_rhs1 = nc.dram_tensor("rhs1", [F_IN, F1 + H1], f16, kind="ExternalInput")
    d_vdst1 = nc.dram_tensor("vdst1", [F_IN, H1], f16, kind="ExternalInput")
    d_rhs2 = nc.dram_tensor("rhs2", [F1, OUT + 2], f16, kind="ExternalInput")
    d_b1c = nc.dram_tensor("b1c", [HID, H1], f32, kind="ExternalInput")
    d_b2r = nc.dram_tensor("b2r", [1, OUT + 2], f16, kind="ExternalInput")
    d_out = nc.dram_tensor("outR", [R, OUT], f32, kind="ExternalOutput")

    with tile.TileContext(nc) as tc:
        with tc.tile_pool(name="const", bufs=1) as const, \
             tc.tile_pool(name="big", bufs=1) as big, \
             tc.tile_pool(name="work", bufs=3) as work, \
             tc.tile_pool(name="qpool", bufs=3) as qpool, \
             tc.tile_pool(name="epool", bufs=3) as epool, \
             tc.tile_pool(name="dram", bufs=1, space="DRAM") as dram, \
             tc.tile_pool(name="ps_mm", bufs=2, space="PSUM") as ps_mm, \
             tc.tile_pool(name="ps_bc", bufs=2, space="PSUM") as ps_bc, \
             tc.tile_pool(name="ps_ag", bufs=2, space="PSUM") as ps_ag, \
             tc.tile_pool(name="ps_sm", bufs=2, space="PSUM") as ps_sm:
            ident = const.tile([128, 128], f32)
            make_identity(nc, ident)
            ident16 = const.tile([128, 128], f16)
            nc.vector.tensor_copy(ident16, ident)
            ones16 = const.tile([1, 128], f16)
            nc.vector.memset(ones16, 1.0)
            ones_col16 = const.tile([128, 1], f16)
            nc.vector.memset(ones_col16, 1.0)
            rhs1_sb = const.tile([128, 2, F1 + H1], f16)
            nc.sync.dma_start(out=rhs1_sb[:, 0, :], in_=d_rhs1[0:128, :])
            nc.sync.dma_start(out=rhs1_sb[:, 1, :], in_=d_rhs1[128:256, :])
            vdst1_sb = const.tile([128, 2, H1], f16)
            nc.sync.dma_start(out=vdst1_sb[:, 0, :], in_=d_vdst1[0:128, :])
            nc.sync.dma_start(out=vdst1_sb[:, 1, :], in_=d_vdst1[128:256, :])
            rhs2_sb = const.tile([128, 4, OUT + 2], f16)
            for kt in range(4):
                nc.sync.dma_start(out=rhs2_sb[:, kt, :],
                                  in_=d_rhs2[kt * 128:(kt + 1) * 128, :])
            b1_sb = const.tile([HID, H1], f32)
            nc.sync.dma_start(out=b1_sb, in_=d_b1c[:, :])
            b2r_sb = const.tile([1, OUT + 2], f16)
            nc.sync.dma_start(out=b2r_sb, in_=d_b2r[:, :])

            # ---- big persistent arrays ----
            adjT_all = big.tile([128, JT, R], f16)       # 32 KB/part
            xT_sb = big.tile([128, 2, N], f16)           # 16 KB/part
            h1_all = big.tile([128, JT, H1, HID + 1], f16)  # 32.5 KB/part
            asrc16 = big.tile([128, JT, H1], f16)
            easrc = big.tile([128, JT, H1], f32)
            e2src = big.tile([128, JT, H1], f32)
            nege2 = big.tile([128, JT, H1], f32)
            adstT = big.tile([H1, R], f32)
            adst_rows = big.tile([1, H1, R], f32)
            adst2T = big.tile([1, R], f32)
            x2T_all = big.tile([128, 4, R], f16)
            u8bc = big.tile([128, 2, R], f16)            # 2-head pipeline
            h2g_all = big.tile([128, N_CORES, IT, G], f32)
            u8bc2 = big.tile([128, R], f16)
            rz2col = big.tile([128, IT], f32)

            nc.vector.memset(h1_all[:, :, :, HID:HID + 1], 1.0)

            # ---- input DMAs: xmT/xT first (a_dst + h1 need them), adjT after
            xmT_sb = big.tile([128, 2, R], f16)
            nc.sync.dma_start(out=xmT_sb[:, 0, :], in_=d_xmT[0:128, :])
            nc.sync.dma_start(out=xmT_sb[:, 1, :], in_=d_xmT[128:256, :])
            for c4 in range(4):
                csl = slice(c4 * (N // 4), (c4 + 1) * (N // 4))
                nc.sync.dma_start(out=xT_sb[:, 0, csl], in_=d_xT16[0:128, csl])
                nc.sync.dma_start(out=xT_sb[:, 1, csl],
                                  in_=d_xT16[128:256, csl])
            for jb in range(JT):
                nc.sync.dma_start(
                    out=adjT_all[:, jb, :],
                    in_=d_adjT[jb * 128:(jb + 1) * 128, :])

            # ---- a_dst (own rows) -> adstT [H1, R] f32 ----
            for it in range(IT):
                ps_ad = ps_sm.tile([128, 128], f32, tag="sm")
                for kb in range(2):
                    nc.tensor.matmul(ps_ad[:, 0:H1],
                                     xmT_sb[:, kb, it * 128:(it + 1) * 128],
                                     vdst1_sb[:, kb, :],
                                     start=(kb == 0), stop=(kb == 1))
                adm = work.tile([128, H1], f32, tag="adm", bufs=2)
                nc.vector.tensor_copy(adm, ps_ad[:, 0:H1])
                ps_adT = ps_sm.tile([128, 128], f32, tag="sm")
                nc.tensor.transpose(ps_adT[0:H1, :], adm, ident)
                nc.vector.tensor_copy(adstT[:, it * 128:(it + 1) * 128],
                                      ps_adT[0:H1, :])
            for h in range(H1):
                nc.sync.dma_start(out=adst_rows[:, h, :], in_=adstT[h:h + 1, :])

            # ---- h1 | a_src per jt ----
            for jt in range(JT):
                cols = slice(jt * 128, (jt + 1) * 128)
                ps_h = ps_mm.tile([128, F1], f32, tag="h")
                ps_ast = ps_sm.tile([128, 128], f32, tag="sm")
                ps_as = ps_ast[:, 0:H1]
                for kb in range(2):
                    nc.tensor.matmul(ps_h, xT_sb[:, kb, cols],
                                     rhs1_sb[:, kb, 0:F1],
                                     start=(kb == 0), stop=(kb == 1))
                    nc.tensor.matmul(ps_as, xT_sb[:, kb, cols],
                                     rhs1_sb[:, kb, F1:F1 + H1],
                                     start=(kb == 0), stop=(kb == 1))
                # h1 [j, h, c] + as slice; split copies across DVE/ACT
                if jt % 2 == 0:
                    nc.vector.tensor_copy(
                        h1_all[:, jt, :, 0:HID],
                        ps_h.rearrange("p (h c) -> p h c", c=HID))
                else:
                    nc.scalar.copy(
                        h1_all[:, jt, :, 0:HID],
                        ps_h.rearrange("p (h c) -> p h c", c=HID))
                nc.scalar.copy(asrc16[:, jt, :], ps_as)
                if jt % 8 == 7:
                    gs = slice(jt - 7, jt + 1)
                    nc.scalar.activation(easrc[:, gs, :], asrc16[:, gs, :],
                                         A.Exp)
                    nc.scalar.activation(e2src[:, gs, :], asrc16[:, gs, :],
                                         A.Exp, scale=NEG_ATT)
                    nc.scalar.activation(nege2[:, gs, :], e2src[:, gs, :],
                                         A.Identity, scale=-1.0)

            # ---- layer-1 attention, head-pipelined ----
            def _pre_head(h):
                u8row = work.tile([1, R], f16, tag="u8row", bufs=2,
                                  name=f"u8r{h}")
                nc.scalar.activation(u8row, adst_rows[:, h, :], A.Exp,
                                     scale=1.0 - NEG_ATT)
                ps_u8 = ps_bc.tile([128, R], f32, tag="bc", name=f"psu8{h}")
                nc.tensor.matmul(ps_u8, ones16, u8row, start=True, stop=True)
                nc.vector.tensor_copy(u8bc[:, h % 2, :], ps_u8)

            _pre_head(0)
            for h in range(H1):
                if h + 1 < H1:
                    _pre_head(h + 1)
                u8 = u8bc[:, h % 2, :]
                ps_agg = ps_ag.tile([HID + 1, R], f32, tag="agg")
                act_qts = [qt for qt in range(QT) if _is_act_quad(h, qt)]
                dve_qts = [qt for qt in range(QT) if not _is_act_quad(h, qt)]
                # ACT-chain q tiles first so the scalar engine runs ahead
                act_q4 = {}
                for qt in act_qts:
                    q4 = qpool.tile([128, 4, R], f16, tag="qa", bufs=3,
                                    name=f"qa{h}_{qt}")
                    for k in range(4):
                        jt = qt * 4 + k
                        r = work.tile([128, R], f16, tag="ract", bufs=3)
                        nc.scalar.activation(
                            r, u8, A.Relu,
                            bias=nege2[:, jt, h:h + 1],
                            scale=easrc[:, jt, h:h + 1])
                        nc.scalar.activation(
                            q4[:, k, :], r, A.Identity,
                            bias=e2src[:, jt, h:h + 1])
                    act_q4[qt] = q4
                n_mm = 0
                for qt in dve_qts + act_qts:
                    if qt in act_q4:
                        q4 = act_q4[qt]
                    else:
                        q4 = qpool.tile([128, 4, R], f16, tag="q")
                        for k in range(4):
                            jt = qt * 4 + k
                            nc.vector.tensor_scalar(
                                q4[:, k, :], u8,
                                easrc[:, jt, h:h + 1],
                                e2src[:, jt, h:h + 1],
                                op0=Al.mult, op1=Al.max)
                    e4 = epool.tile([128, 4, R], f16, tag="e")
                    nc.vector.tensor_tensor(
                        e4.rearrange("p a b -> p (a b)"),
                        q4.rearrange("p a b -> p (a b)"),
                        adjT_all[:, qt * 4:(qt + 1) * 4, :].rearrange(
                            "p a b -> p (a b)"),
                        op=Al.min)
                    for k in range(4):
                        jt = qt * 4 + k
                        nc.tensor.matmul(ps_agg, h1_all[:, jt, h, :],
                                         e4[:, k, :],
                                         start=(n_mm == 0),
                                         stop=(n_mm == JT - 1))
                        n_mm += 1
                rz = work.tile([1, R], f16, tag="rz", bufs=2)
                with nc.allow_low_precision(reason="1/z in fp16: 1e-3 rel ok"):
                    nc.vector.reciprocal(rz, ps_agg[HID:HID + 1, :])
                ps_rzb = ps_bc.tile([128, R], f32, tag="bc")
                nc.tensor.matmul(ps_rzb[0:HID, :], ones16[:, 0:HID], rz,
                                 start=True, stop=True)
                rzb_sb = work.tile([HID, R], f16, tag="rzb", bufs=2)
                nc.scalar.copy(rzb_sb, ps_rzb[0:HID, :])
                y_h = work.tile([HID, R], f16, tag="yh", bufs=2)
                nc.vector.tensor_mul(y_h, ps_agg[0:HID, :], rzb_sb)
                po = (h % 2) * HID
                nc.scalar.activation(
                    x2T_all[po:po + HID, h // 2, :], y_h, A.Prelu,
                    bias=b1_sb[:, h:h + 1], alpha=NEG_OUT)

            # ---- layer 2: h2 per it, bounce, AllGather ----
            bounce_in = dram.tile([R, G], f32, name="bin")
            bounce_out = dram.tile([N_CORES, R, G], f32,
                                   addr_space="Shared", name="bout")
            for it in range(IT):
                ps_h2t = ps_mm.tile([128, R], f32, tag="h")
                ps_h2 = ps_h2t[:, 0:OUT + 2]
                for kt in range(4):
                    nc.tensor.matmul(
                        ps_h2,
                        x2T_all[:, kt, it * 128:(it + 1) * 128],
                        rhs2_sb[:, kt, :],
                        start=(kt == 0), stop=False)
                nc.tensor.matmul(ps_h2, ones16, b2r_sb,
                                 start=False, stop=True)
                h2m = work.tile([128, G], f32, tag="h2m", bufs=2)
                nc.vector.tensor_copy(h2m[:, 0:HID].bitcast(f16),
                                      ps_h2[:, 0:OUT])
                nc.scalar.activation(h2m[:, HID:HID + 1],
                                     ps_h2[:, OUT:OUT + 1], A.Exp)
                nc.scalar.activation(h2m[:, HID + 1:HID + 2],
                                     ps_h2[:, OUT:OUT + 1], A.Exp,
                                     scale=NEG_ATT)
                nc.sync.dma_start(
                    out=bounce_in[it * 128:(it + 1) * 128, :], in_=h2m)
                ad2m = work.tile([128, 1], f32, tag="ad2m", bufs=2)
                nc.scalar.copy(ad2m, ps_h2[:, OUT + 1:OUT + 2])
                ps_adT2 = ps_sm.tile([1, 128], f32, tag="sm")
                nc.tensor.transpose(ps_adT2, ad2m, ident)
                nc.vector.tensor_copy(adst2T[:, it * 128:(it + 1) * 128],
                                      ps_adT2)
            nc.gpsimd.collective_compute(
                "AllGather",
                bass.mybir.AluOpType.bypass,
                replica_groups=[list(range(N_CORES))],
                ins=[bounce_in.opt()],
                outs=[bounce_out.opt()],
            )
            for c8 in range(N_CORES):
                nc.sync.dma_start(
                    out=h2g_all[:, c8, :, :],
                    in_=bounce_out[c8].rearrange("(r1 p) g -> p r1 g", p=128))

            # ---- layer-2 attention ----
            u8row2 = work.tile([1, R], f16, tag="u8row", bufs=2)
            nc.scalar.activation(u8row2, adst2T, A.Exp, scale=1.0 - NEG_ATT)
            ps_u82 = ps_bc.tile([128, R], f32, tag="bc")
            nc.tensor.matmul(ps_u82, ones16, u8row2, start=True, stop=True)
            nc.vector.tensor_copy(u8bc2, ps_u82)

            # reuse L1 pools: o2 in ps_mm "h" shape, z2 rides an "agg" buffer
            ps_o2 = ps_mm.tile([128, R], f32, tag="h")
            ps_z2t = ps_ag.tile([HID + 1, R], f32, tag="agg")
            ps_z2 = ps_z2t[HID:HID + 1, :]
            for qt in range(QT):
                q4 = qpool.tile([128, 4, R], f16, tag="q")
                for k in range(4):
                    jt = qt * 4 + k
                    c8, r1 = jt // IT, jt % IT
                    nc.vector.tensor_scalar(
                        q4[:, k, :], u8bc2,
                        h2g_all[:, c8, r1, HID:HID + 1],
                        h2g_all[:, c8, r1, HID + 1:HID + 2],
                        op0=Al.mult, op1=Al.max)
                e4 = epool.tile([128, 4, R], f16, tag="e")
                nc.vector.tensor_tensor(
                    e4.rearrange("p a b -> p (a b)"),
                    q4.rearrange("p a b -> p (a b)"),
                    adjT_all[:, qt * 4:(qt + 1) * 4, :].rearrange(
                        "p a b -> p (a b)"),
                    op=Al.min)
                for k in range(4):
                    jt = qt * 4 + k
                    c8, r1 = jt // IT, jt % IT
                    nc.tensor.matmul(
                        ps_o2, h2g_all[:, c8, r1, 0:HID].bitcast(f16),
                        e4[:, k, :],
                        start=(jt == 0), stop=(jt == JT - 1))
                    nc.tensor.matmul(
                        ps_z2, ones_col16, e4[:, k, :],
                        start=(jt == 0), stop=(jt == JT - 1))
            # per-it: transpose z2 chunk -> recip col; transpose o2 -> prelu
            o2sb = work.tile([128, R], f32, tag="o2sb", bufs=1)
            for it in range(IT):
                nc.scalar.copy(o2sb[:, it * 128:(it + 1) * 128],
                               ps_o2[:, it * 128:(it + 1) * 128])
            z2sb = work.tile([1, R], f16, tag="z2sb", bufs=1)
            nc.vector.tensor_copy(z2sb, ps_z2)
            del ps_z2t
            outT_sb = work.tile([128, IT, OUT], f32, tag="outT", bufs=1)
            for it in range(IT):
                isl = slice(it * 128, (it + 1) * 128)
                ps_zTt = ps_sm.tile([128, 128], f32, tag="sm")
                ps_zT = ps_zTt.bitcast(f16)[:, 0:1]
                nc.tensor.transpose(ps_zT, z2sb[:, isl], ident16[0:1, 0:1])
                with nc.allow_low_precision(reason="1/z2 col fp16 src ok"):
                    nc.vector.reciprocal(rz2col[:, it:it + 1], ps_zT)
                ps_oT = ps_sm.tile([128, 128], f32, tag="sm")
                nc.tensor.transpose(ps_oT, o2sb[:, isl], ident)
                nc.scalar.activation(outT_sb[:, it, :], ps_oT, A.Prelu,
                                     scale=rz2col[:, it:it + 1], alpha=NEG_OUT)
            nc.sync.dma_start(
                out=d_out.rearrange("(i p) c -> p i c", p=128), in_=outT_sb)

    nc.finalize()
    return nc


def _prep_host(x, adj, w1, att_src1, att_dst1, b1, w2, att_src2, att_dst2, b2):
    x = np.asarray(x, np.float32).reshape(N, F_IN)
    adj = np.asarray(adj, np.float32).reshape(N, N)
    w1 = np.asarray(w1, np.float32)
    w2 = np.asarray(w2, np.float32)
    att_src1 = np.asarray(att_src1, np.float32)
    att_dst1 = np.asarray(att_dst1, np.float32)
    att_src2 = np.asarray(att_src2, np.float32)
    att_dst2 = np.asarray(att_dst2, np.float32)
    b1 = np.asarray(b1, np.float32)
    b2 = np.asarray(b2, np.float32)

    xT = np.ascontiguousarray(x.T)
    xT16 = xT.astype(np.float16)
    adjm = (adj * MASKV).astype(np.float16)
    v_src1 = np.empty((F_IN, H1), np.float32)
    v_dst1 = np.empty((F_IN, H1), np.float32)
    for h in range(H1):
        blk = w1[:, h * HID:(h + 1) * HID]
        v_src1[:, h] = blk @ att_src1[h]
        v_dst1[:, h] = blk @ att_dst1[h]
    rhs1 = np.ascontiguousarray(
        np.concatenate([w1, v_src1], axis=1)).astype(np.float16)
    v_src2 = (w2 @ att_src2[0])[:, None]
    v_dst2 = (w2 @ att_dst2[0])[:, None]
    rhs2 = np.ascontiguousarray(
        np.concatenate([w2, v_src2, v_dst2], axis=1)).astype(np.float16)
    b1c = np.ascontiguousarray(b1.reshape(H1, HID).T)
    b2r = np.zeros((1, OUT + 2), np.float16)
    b2r[0, 0:OUT] = b2

    in_maps = []
    for c in range(N_CORES):
        rows = slice(c * R, (c + 1) * R)
        in_maps.append({
            "xT16": xT16,
            "xmT": np.ascontiguousarray(xT16[:, rows]),
            "adjT": np.ascontiguousarray(adjm[rows, :].T),
            "rhs1": rhs1,
            "vdst1": v_dst1.astype(np.float16),
            "rhs2": rhs2,
            "b1c": b1c,
            "b2r": b2r,
        })
    return in_maps


def kernel(**inputs) -> np.ndarray:
    from concourse.bass_utils import run_bass_kernel_spmd

    if "nc" not in _CACHE:
        _CACHE["nc"] = _build()
    nc = _CACHE["nc"]
    in_maps = _prep_host(**inputs)
    try:
        res = run_bass_kernel_spmd(nc, in_maps, list(range(N_CORES)))
    except Exception:
        # transient NRT device wedge — one clean retry
        res = run_bass_kernel_spmd(nc, in_maps, list(range(N_CORES)))
    out = np.empty((1, N, OUT), np.float32)
    for c in range(N_CORES):
        out[0, c * R:(c + 1) * R, :] = res.results[c]["outR"]
    return out
